# revision 12
# baseline (speedup 1.0000x reference)
"""Trainium2 Bass kernel for nn_CCPL_14216341750304 (CCPL / PatchNCE loss).

Math (per batch b, one per NeuronCore, 8 cores):
    g_c = f[b][:, c_ids], g_n = f[b][:, n_ids]      # gather, both q and k
    d   = g_c - g_n                                  # [S, 128] (q64 | k64)
    H   = relu(d @ blockdiag(W0, W0) + b0)           # MLP layer 1
    E   = H @ W1 + b1                                # [S, 16] per tensor
    F   = E / (||E||_2 + eps)                        # L2 norm over 16 ch
    M   = Fq.T @ Fk   [S, S]                         # cosine sims, |M| <= 1
    loss_row s = 1/tau + log(sum_t exp((M[s,t]-1)/tau)) - M[s,s]/tau
l_pos is exactly diag(M); masking the diag with -inf and concatenating
l_pos yields the same logsumexp multiset as the unmasked row.  |M|<=1
lets a constant shift of 1 replace the row-max (no overflow, no masking).

Key layout choice: the host stages the features TRANSPOSED and
PAIR-PACKED as fst2[pid] = [fq|fk of pixel 2*pid, fq|fk of 2*pid+1],
[HW/2, 256] fp16 — one 512B DRAM row per pixel pair, so pair ids fit
int16 (< 32768) and a sample gather is one contiguous row read.  The
device gathers ONLY the sampled rows (~1.2 MB) via batched dma_gather
ops spread over 4 SWDGE queues instead of streaming + transforming the
full 32 MB map; the odd/even pixel is selected post-gather with
copy_predicated on the parity bit.  W0 is applied after the gather
(linearity: W0 commutes with the diff).  All network ops (gather,
diff, MLP, normalize, NCE, reduction) run on device.

c_ids is tile(centers, 8) in the reference sampler, so only 256 unique
center rows (2 blocks) are gathered; the host verifies this structure
and falls back to a generic 2048-row variant otherwise.

MLP layer 2 + L2 normalize run in the TRANSPOSED orientation
(E^T tiles [128 samples, 32 ch]) so elementwise/reduce work uses all
128 lanes instead of 16; the normalized embeddings are PE-transposed
back to [16, S] for the NCE matmuls.

NCE: 16 M-chunks of [128, 2048] PSUM; exp+rowsum fused on ScalarE
(accum_out); PSUM drain split between ScalarE (direct, f32) and a
double-buffered VectorE bf16 staging copy so the two engines overlap.
Output [1, 2] per core: [sum_s log(rowsum_s), sum_s l_pos_s].
Host: loss = sum_cores(S/tau + o0 - o1/tau) / (8*S).
"""

import numpy as np

import concourse.bacc as bacc
import concourse.bass as bass
import concourse.mybir as mybir
import concourse.tile as tile
from concourse import bass_utils
from concourse.bass import ds, ts

F32 = mybir.dt.float32
F16 = mybir.dt.float16
BF16 = mybir.dt.bfloat16
I32 = mybir.dt.int32
I16 = mybir.dt.int16
I8 = mybir.dt.int8

B, C, H, W = 8, 64, 256, 256
HW = H * W                 # 65536
S = 2048                   # samples per batch (8*256)
NJ = S // 128              # 16 gather blocks per id set
NU = 256                   # unique centers when c_ids = tile(c, 8)
TAU = 0.07
EPS = 1e-7
NCORES = 8
EXPBIAS = -1.0 / TAU       # exp((M-1)/tau) = exp(M*(1/tau) + (-1/tau))

_CACHE = {}


def _build(n_bodies=1, stop_after=None, loop_n=0, generic_c=False,
           b1_nonzero=False):
    """Build + compile the per-core Bass program (cached).

    stop_after in {"gather", "transform", "mlp"} truncates the body.
    loop_n > 0 wraps the body in a device-side For loop (perf
    amplification); constants are hoisted out of the loop.
    generic_c: don't assume c_ids = tile(c[:256], 8).
    b1_nonzero: emit the b1 bias-init matmul (b1 is zeros otherwise).
    """
    key = f"nc{n_bodies}_{stop_after}_{loop_n}_{generic_c}_{b1_nonzero}"
    if key in _CACHE:
        return _CACHE[key]

    nc = bacc.Bacc("TRN2", target_bir_lowering=False, debug=False,
                   num_swdge_queues=4)

    def dram_in(name, shape, dt):
        return nc.dram_tensor(name, shape, dt, kind="ExternalInput").ap()

    ncj = NJ if generic_c else 2
    d = {
        "fst": dram_in("fst", [HW // 2, 256], F16),  # pair rows, 512B
        "idxc": dram_in("idxc", [128, ncj * 8], I16),  # wrapped pair ids
        "idxn": dram_in("idxn", [128, NJ * 8], I16),
        "maskc": dram_in("maskc", [128, ncj], I8),   # odd-parity per sample
        "maskn": dram_in("maskn", [128, NJ], I8),
        "wblk": dram_in("wblk", [128, 128], BF16),  # blockdiag(W0, W0)
        "w1qk": dram_in("w1qk", [128, 32], BF16),   # [W1q-pad | W1k-pad]
        "b0b": dram_in("b0b", [128, 1], F32),       # [b0; b0]
        "ident": dram_in("ident", [128, 128], F16),
        "identb": dram_in("identb", [128, 128], BF16),
        "ones128": dram_in("ones128", [128, 1], F32),
    }
    if b1_nonzero:
        d["onessq"] = dram_in("onessq", [128, 128], BF16)
        d["b1w"] = dram_in("b1w", [128, 512], BF16)  # b1 pattern / 128
    out_d = nc.dram_tensor("out", [1, 2], F32, kind="ExternalOutput").ap()

    AF = mybir.ActivationFunctionType

    with tile.TileContext(nc) as tc:
        with tc.tile_pool(name="const", bufs=1) as cp:
            ct = {}
            for name, ap_ in d.items():
                if name == "fst":
                    continue
                t = cp.tile(list(ap_.shape), ap_.dtype, tag=f"c_{name}")
                nc.sync.dma_start(t[:], ap_)
                ct[name] = t
            ebias = cp.tile([128, 1], F32)
            nc.gpsimd.memset(ebias[:], EXPBIAS)
            ct["ebias"] = ebias

            if loop_n:
                with tc.For_i(0, loop_n, 1):
                    _emit_body(nc, tc, 0, AF, d["fst"], ct, out_d,
                               generic_c, b1_nonzero, stop_after)
            else:
                for _body_i in range(n_bodies):
                    _emit_body(nc, tc, _body_i, AF, d["fst"], ct, out_d,
                               generic_c, b1_nonzero, stop_after)

    nc.compile()
    _CACHE[key] = nc
    return nc


def _emit_body(nc, tc, uid, AF, fst_d, ct, out_d, generic_c, b1_nonzero,
               stop_after=None):
        idxc, idxn = ct["idxc"], ct["idxn"]
        maskc, maskn = ct["maskc"], ct["maskn"]
        wblk, w1qk, b0b = ct["wblk"], ct["w1qk"], ct["b0b"]
        ident, identb, ones128 = ct["ident"], ct["identb"], ct["ones128"]
        ebias = ct["ebias"]
        with (
            # bufs=2: double-buffer work tiles so iteration i+1's gathers
            # and head phases overlap iteration i's NCE tail
            tc.tile_pool(name=f"work{uid}", bufs=2) as wp,
            tc.tile_pool(name=f"psum{uid}", bufs=2,
                         space=bass.MemorySpace.PSUM) as pp,
        ):
            # ---- gathers: dma_gather of 512B pair rows, int16 pair ids,
            # sample i lands at [i % 128, i // 128, :].  n split over SWDGE
            # queues 0/2/3, c on queue 1.
            ncj = NJ if generic_c else 2
            gn = wp.tile([128, NJ * 256], F16)
            gn3 = gn[:].rearrange("p (j e) -> p j e", e=256)
            for js, je, q in ((0, 6, 0), (6, 11, 2), (11, 16, 3)):
                ni = (je - js) * 128
                nc.gpsimd.dma_gather(
                    gn3[:, js:je, :], fst_d,
                    idxn[:, js * 8:js * 8 + ni // 16],
                    ni, ni, 256, queue_num=q,
                )
            gc = wp.tile([128, ncj * 256], F16)
            gc3 = gc[:].rearrange("p (j e) -> p j e", e=256)
            nc.gpsimd.dma_gather(
                gc3, fst_d, idxc[:], ncj * 128, ncj * 128, 256, queue_num=1,
            )

            if stop_after == "gather":
                dummy = wp.tile([1, 2], F32)
                nc.vector.tensor_copy(dummy[:], gn[0:1, 0:2])
                nc.sync.dma_start(out_d, dummy[:])
                return

            # ---- parity select + diff (s-rows orientation, fp16) ----
            for j in range(ncj):
                mc = maskc[:, j:j + 1].to_broadcast([128, 128])
                nc.vector.copy_predicated(
                    gc3[:, j, 0:128], mc, gc3[:, j, 128:256])
            djall = wp.tile([128, S], F16)
            for j in range(NJ):
                mn = maskn[:, j:j + 1].to_broadcast([128, 128])
                nc.vector.copy_predicated(
                    gn3[:, j, 0:128], mn, gn3[:, j, 128:256])
                jc = j if generic_c else j % 2
                nc.vector.tensor_sub(
                    djall[:, ts(j, 128)], gc3[:, jc, 0:128], gn3[:, j, 0:128]
                )

            # ---- transpose diff blocks to [128ch, S] ----
            hin = wp.tile([128, S], BF16)
            for j2 in range(NJ // 2):
                pst = pp.tile([128, 256], F16, tag="ps")
                for h in range(2):
                    nc.tensor.transpose(
                        out=pst[:, ts(h, 128)],
                        in_=djall[:, ts(2 * j2 + h, 128)], identity=ident[:]
                    )
                if j2 % 2 == 0:
                    nc.vector.tensor_copy(hin[:, ts(j2, 256)], pst[:])
                else:
                    nc.scalar.copy(hin[:, ts(j2, 256)], pst[:])

            # ---- W0 matmul + relu (bias b0) ----
            hid = wp.tile([128, S], BF16)
            for j in range(4):
                psH = pp.tile([128, 512], F32, tag="ps")
                nc.tensor.matmul(
                    out=psH[:],
                    lhsT=wblk[:],
                    rhs=hin[:, ts(j, 512)],
                    start=True,
                    stop=True,
                )
                nc.scalar.activation(
                    hid[:, ts(j, 512)], psH[:], AF.Relu, bias=b0b[:, 0:1]
                )

            if stop_after == "transform":
                dummy = wp.tile([1, 2], F32)
                nc.vector.tensor_copy(dummy[:], hid[0:1, 0:2])
                nc.sync.dma_start(out_d, dummy[:])
                return

            # ---- MLP layer 2 + L2 normalize, transposed orientation ----
            # E^T tiles: [128 samples, 32] = [Eq^T | Ek^T] per 128-sample
            # block, all 16 blocks packed in one PSUM bank [128, 512].
            psET = pp.tile([128, 512], F32, tag="ps")
            if b1_nonzero:
                nc.tensor.matmul(
                    out=psET[:], lhsT=ct["onessq"][:], rhs=ct["b1w"][:],
                    start=True, stop=False,
                )
            for t in range(NJ):
                nc.tensor.matmul(
                    out=psET[:, ts(t, 32)],
                    lhsT=hid[:, ts(t, 128)],
                    rhs=w1qk[:],
                    start=not b1_nonzero,
                    stop=True,
                )
            et = wp.tile([128, 512], F32)
            nc.vector.tensor_copy(et[:], psET[:])
            sq = wp.tile([128, 512], F32)
            nc.vector.tensor_mul(sq[:], et[:], et[:])
            ss = wp.tile([128, 32], F32)
            nc.vector.tensor_reduce(
                ss[:].rearrange("p (t u) -> p t u", u=1),
                sq[:].rearrange("p (t c) -> p t c", c=16),
                axis=mybir.AxisListType.X, op=mybir.AluOpType.add,
            )
            nrm = wp.tile([128, 32], F32)
            nc.scalar.activation(nrm[:], ss[:], AF.Sqrt)
            nrme = wp.tile([128, 32], F32)
            nc.vector.tensor_scalar_add(nrme[:], nrm[:], EPS)
            inv = wp.tile([128, 32], F32)
            nc.vector.reciprocal_approx_fast(inv[:], nrme[:])
            fT = wp.tile([128, 512], BF16)
            nc.vector.tensor_mul(
                fT[:].rearrange("p (t c) -> p t c", c=16),
                et[:].rearrange("p (t c) -> p t c", c=16),
                inv[:].to_broadcast([128, 32, 16]),
            )

            # ---- l_pos partials: sum_c Fq*Fk per sample ----
            fT4 = fT[:].rearrange("p (t two c) -> p t two c", two=2, c=16)
            prod = wp.tile([128, 256], F32)
            nc.vector.tensor_mul(
                prod[:].rearrange("p (t c) -> p t c", c=16),
                fT4[:, :, 0, :], fT4[:, :, 1, :],
            )
            lpost = wp.tile([128, 16], F32)
            nc.vector.tensor_reduce(
                lpost[:].rearrange("p (t u) -> p t u", u=1),
                prod[:].rearrange("p (t c) -> p t c", c=16),
                axis=mybir.AxisListType.X, op=mybir.AluOpType.add,
            )
            lred = wp.tile([128, 2], F32)
            nc.vector.tensor_reduce(
                lred[:, 1:2], lpost[:],
                axis=mybir.AxisListType.X, op=mybir.AluOpType.add,
            )

            # ---- transpose F^T back to [16, S] bf16 for the NCE ----
            fqb = wp.tile([16, S], BF16)
            fkb = wp.tile([16, S], BF16)
            for half, fb in ((0, fqb), (1, fkb)):
                psF = pp.tile([16, S], BF16, tag="ps")
                for t in range(NJ):
                    nc.tensor.transpose(
                        out=psF[:, ts(t, 128)],
                        in_=fT[:, ds(t * 32 + half * 16, 16)],
                        identity=identb[:],
                    )
                if half == 0:
                    nc.vector.tensor_copy(fb[:], psF[:])
                else:
                    nc.scalar.copy(fb[:], psF[:])

            out_sb = wp.tile([1, 2], F32)

            if stop_after == "mlp":
                nc.vector.tensor_copy(out_sb[:], fqb[0:1, 0:2])
                nc.sync.dma_start(out_d, out_sb[:])
                return

            # ---- NCE: 32 half-chunks [128, 1024] of M, exp+rowsum fused.
            # Dedicated PSUM tag "pm" so the next iteration's head PSUM use
            # (tag "ps") doesn't wait behind the NCE drain; halved chunks
            # give finer ACT/DVE alternation.
            rowsums = wp.tile([128, 32], F32)
            escr = wp.tile([128, 1024], BF16)
            for h in range(32):
                i, cc = h // 2, h % 2
                psM = pp.tile([128, 1024], F32, tag="pm")
                for j in range(2):
                    nc.tensor.matmul(
                        out=psM[:, ts(j, 512)],
                        lhsT=fqb[:, ts(i, 128)],
                        rhs=fkb[:, ds(cc * 1024 + j * 512, 512)],
                        start=True,
                        stop=True,
                    )
                if h % 16 in (0, 3, 6, 9, 12):  # 10 direct, 22 offloaded
                    # direct: ACT reads PSUM f32 (1x)
                    nc.scalar.activation(
                        escr[:], psM[:], AF.Exp,
                        bias=ebias[:, 0:1], scale=1.0 / TAU,
                        accum_out=rowsums[:, h:h + 1],
                    )
                else:
                    # offload PSUM read to DVE; ACT exp runs 2x from bf16
                    # SBUF; double-buffered so DVE copy h+1 overlaps exp h
                    msb = wp.tile([128, 1024], BF16, tag=f"msb{h % 2}")
                    nc.vector.tensor_copy(msb[:], psM[:])
                    nc.scalar.activation(
                        escr[:], msb[:], AF.Exp,
                        bias=ebias[:, 0:1], scale=1.0 / TAU,
                        accum_out=rowsums[:, h:h + 1],
                    )

            # row-chunk sums = pairs of half-chunk sums
            rs16 = wp.tile([128, 16], F32)
            nc.vector.tensor_reduce(
                rs16[:].rearrange("p (t u) -> p t u", u=1),
                rowsums[:].rearrange("p (t u) -> p t u", u=2),
                axis=mybir.AxisListType.X, op=mybir.AluOpType.add,
            )
            logt = wp.tile([128, 16], F32)
            nc.scalar.activation(logt[:], rs16[:], AF.Ln)
            nc.vector.tensor_reduce(
                lred[:, 0:1], logt[:], axis=mybir.AxisListType.X,
                op=mybir.AluOpType.add,
            )
            psS = pp.tile([1, 2], F32, tag="ps")
            nc.tensor.matmul(
                out=psS[:], lhsT=ones128[:], rhs=lred[:], start=True, stop=True
            )
            nc.vector.tensor_copy(out_sb[:], psS[:])
            nc.sync.dma_start(out_d, out_sb[:])


def _host_prep(f_q, f_k, W0, b0, W1, b1, c_ids, n_ids):
    """Build the per-core input maps (host-side sharding + layout prep)."""
    f_q = np.asarray(f_q, dtype=np.float32).reshape(B, C, HW)
    f_k = np.asarray(f_k, dtype=np.float32).reshape(B, C, HW)
    W0 = np.asarray(W0, dtype=np.float32)
    b0 = np.asarray(b0, dtype=np.float32)
    W1 = np.asarray(W1, dtype=np.float32)
    b1 = np.asarray(b1, dtype=np.float32)
    c_ids = np.asarray(c_ids).astype(np.int64)
    n_ids = np.asarray(n_ids).astype(np.int64)

    generic_c = not np.array_equal(np.tile(c_ids[:NU], 8), c_ids)
    b1_nonzero = bool(np.any(b1 != 0))

    import ml_dtypes
    bf = ml_dtypes.bfloat16
    wblk = np.zeros((128, 128), np.float32)
    wblk[0:64, 0:64] = W0
    wblk[64:128, 64:128] = W0
    wblk = wblk.astype(bf)
    w1qk = np.zeros((128, 32), np.float32)
    w1qk[0:64, 0:16] = W1
    w1qk[64:128, 16:32] = W1
    w1qk = w1qk.astype(bf)
    b0b = np.concatenate([b0, b0]).reshape(128, 1).astype(np.float32)

    def wrap16(ids):
        # dma_gather idx layout: idxs[p, s] = pair_id[s*16 + p] for p < 16,
        # replicated across the 8 partition groups of 16
        w = (ids >> 1).astype(np.int16).reshape(-1, 16).T
        return np.tile(w, (8, 1)).copy()

    def parity(ids, nj):
        # m[p, j] = odd-parity of sample s = j*128 + p
        return (ids & 1).astype(np.int8).reshape(nj, 128).T.copy()

    c_eff = c_ids if generic_c else c_ids[:NU]
    common = {
        "wblk": wblk, "w1qk": w1qk, "b0b": b0b,
        "ones128": np.ones((128, 1), np.float32),
        "ident": np.eye(128, dtype=np.float16),
        "identb": np.eye(128, dtype=np.float32).astype(bf),
        "idxn": wrap16(n_ids), "idxc": wrap16(c_eff),
        "maskn": parity(n_ids, NJ),
        "maskc": parity(c_eff, NJ if generic_c else 2),
    }
    if b1_nonzero:
        common["onessq"] = np.ones((128, 128), np.float32).astype(bf)
        b1p = np.zeros((32,), np.float32)
        b1p[0:16] = b1
        b1p[16:32] = b1
        common["b1w"] = np.tile(b1p / 128.0, 16).reshape(1, 512).repeat(
            128, axis=0).astype(bf)

    in_maps = []
    for b in range(B):
        m = dict(common)
        # [HW/2, 256] fp16: row pid = [fq|fk of px 2*pid, fq|fk of
        # 2*pid+1] — one 512B row per pixel pair (pair id fits int16).
        fst = np.empty((HW, 128), np.float16)
        fst[:, 0:64] = f_q[b].T
        fst[:, 64:128] = f_k[b].T
        m["fst"] = fst.reshape(HW // 2, 256)
        in_maps.append(m)
    return in_maps, generic_c, b1_nonzero


def _finish(results):
    total = 0.0
    for r in results:
        o = np.asarray(r["out"], dtype=np.float64).reshape(2)
        total += S / TAU + o[0] - o[1] / TAU
    return np.float32(total / (B * S))


def kernel(**inputs) -> np.ndarray:
    in_maps, generic_c, b1_nonzero = _host_prep(
        inputs["f_q"], inputs["f_k"], inputs["W0"], inputs["b0"],
        inputs["W1"], inputs["b1"], inputs["c_ids"], inputs["n_ids"],
    )
    nc = _build(generic_c=generic_c, b1_nonzero=b1_nonzero)
    res = bass_utils.run_bass_kernel_spmd(
        nc, in_maps, core_ids=list(range(NCORES))
    )
    return _finish(res.results)


# revision 16
# speedup vs baseline: 1.0915x; 1.0915x over previous
"""Trainium2 Bass kernel for nn_CCPL_14216341750304 (CCPL / PatchNCE loss).

Math (per batch b, one per NeuronCore, 8 cores):
    g_c = f[b][:, c_ids], g_n = f[b][:, n_ids]      # gather, both q and k
    d   = g_c - g_n                                  # [S, 128] (q64 | k64)
    H   = relu(d @ blockdiag(W0, W0) + b0)           # MLP layer 1
    E   = H @ W1 + b1                                # [S, 16] per tensor
    F   = E / (||E||_2 + eps)                        # L2 norm over 16 ch
    M   = Fq.T @ Fk   [S, S]                         # cosine sims, |M| <= 1
    loss_row s = 1/tau + log(sum_t exp((M[s,t]-1)/tau)) - M[s,s]/tau
l_pos is exactly diag(M); masking the diag with -inf and concatenating
l_pos yields the same logsumexp multiset as the unmasked row.  |M|<=1
lets a constant shift of 1 replace the row-max (no overflow, no masking).

Key layout choice: the host stages the features TRANSPOSED and
PAIR-PACKED as fst2[pid] = [fq|fk of pixel 2*pid, fq|fk of 2*pid+1],
[HW/2, 256] fp16 — one 512B DRAM row per pixel pair, so pair ids fit
int16 (< 32768) and a sample gather is one contiguous row read.  The
device gathers ONLY the sampled rows (~1.2 MB) via batched dma_gather
ops spread over 4 SWDGE queues instead of streaming + transforming the
full 32 MB map; the odd/even pixel is selected post-gather with
copy_predicated on the parity bit.  W0 is applied after the gather
(linearity: W0 commutes with the diff).  All network ops (gather,
diff, MLP, normalize, NCE, reduction) run on device.

c_ids is tile(centers, 8) in the reference sampler, so only 256 unique
center rows (2 blocks) are gathered; the host verifies this structure
and falls back to a generic 2048-row variant otherwise.

MLP layer 2 + L2 normalize run in the TRANSPOSED orientation
(E^T tiles [128 samples, 32 ch]) so elementwise/reduce work uses all
128 lanes instead of 16; the normalized embeddings are PE-transposed
back to [16, S] for the NCE matmuls.

NCE: 16 M-chunks of [128, 2048] PSUM; exp+rowsum fused on ScalarE
(accum_out); PSUM drain split between ScalarE (direct, f32) and a
double-buffered VectorE bf16 staging copy so the two engines overlap.
Output [1, 2] per core: [sum_s log(rowsum_s), sum_s l_pos_s].
Host: loss = sum_cores(S/tau + o0 - o1/tau) / (8*S).
"""

import numpy as np

import concourse.bacc as bacc
import concourse.bass as bass
import concourse.mybir as mybir
import concourse.tile as tile
from concourse import bass_utils
from concourse.bass import ds, ts

F32 = mybir.dt.float32
F16 = mybir.dt.float16
BF16 = mybir.dt.bfloat16
I32 = mybir.dt.int32
I16 = mybir.dt.int16
I8 = mybir.dt.int8

B, C, H, W = 8, 64, 256, 256
HW = H * W                 # 65536
S = 2048                   # samples per batch (8*256)
NJ = S // 128              # 16 gather blocks per id set
NU = 256                   # unique centers when c_ids = tile(c, 8)
TAU = 0.07
EPS = 1e-7
NCORES = 8
EXPBIAS = -1.0 / TAU       # exp((M-1)/tau) = exp(M*(1/tau) + (-1/tau))

_CACHE = {}


def _build(n_bodies=1, stop_after=None, loop_n=0, generic_c=False,
           b1_nonzero=False):
    """Build + compile the per-core Bass program (cached).

    stop_after in {"gather", "transform", "mlp"} truncates the body.
    loop_n > 0 wraps the body in a device-side For loop (perf
    amplification); constants are hoisted out of the loop.
    generic_c: don't assume c_ids = tile(c[:256], 8).
    b1_nonzero: emit the b1 bias-init matmul (b1 is zeros otherwise).
    """
    key = f"nc{n_bodies}_{stop_after}_{loop_n}_{generic_c}_{b1_nonzero}"
    if key in _CACHE:
        return _CACHE[key]

    nc = bacc.Bacc("TRN2", target_bir_lowering=False, debug=False,
                   num_swdge_queues=4)

    def dram_in(name, shape, dt):
        return nc.dram_tensor(name, shape, dt, kind="ExternalInput").ap()

    ncj = NJ if generic_c else 2
    d = {
        "fst": dram_in("fst", [HW // 2, 256], F16),  # pair rows, 512B
        "idxc": dram_in("idxc", [128, ncj * 8], I16),  # wrapped pair ids
        "idxn": dram_in("idxn", [128, NJ * 8], I16),
        "maskc": dram_in("maskc", [128, ncj], I8),   # odd-parity per sample
        "maskn": dram_in("maskn", [128, NJ], I8),
        "wblk": dram_in("wblk", [128, 128], BF16),  # blockdiag(W0, W0)
        "w1qk": dram_in("w1qk", [128, 32], BF16),   # [W1q-pad | W1k-pad]
        "b0b": dram_in("b0b", [128, 1], F32),       # [b0; b0]
        "ident": dram_in("ident", [128, 128], F16),
        "identb": dram_in("identb", [128, 128], BF16),
        "ones128": dram_in("ones128", [128, 1], F32),
    }
    if b1_nonzero:
        d["onessq"] = dram_in("onessq", [128, 128], BF16)
        d["b1w"] = dram_in("b1w", [128, 512], BF16)  # b1 pattern / 128
    out_d = nc.dram_tensor("out", [1, 2], F32, kind="ExternalOutput").ap()

    AF = mybir.ActivationFunctionType

    with tile.TileContext(nc) as tc:
        with tc.tile_pool(name="const", bufs=1) as cp:
            ct = {}
            for name, ap_ in d.items():
                if name == "fst":
                    continue
                t = cp.tile(list(ap_.shape), ap_.dtype, tag=f"c_{name}")
                nc.sync.dma_start(t[:], ap_)
                ct[name] = t
            ebias = cp.tile([128, 1], F32)
            nc.gpsimd.memset(ebias[:], EXPBIAS)
            ct["ebias"] = ebias

            if loop_n:
                # 2x-unrolled loop body with per-body work pools and a
                # SHARED psum pool: body u+1's head genuinely overlaps
                # body u's NCE (tile allocation is per emission site, so a
                # single body in the loop reuses one tile set and
                # serializes iterations regardless of pool bufs).
                with tc.tile_pool(name="psum_sh", bufs=2,
                                  space=bass.MemorySpace.PSUM) as pp_sh:
                    with tc.For_i(0, loop_n // 2, 1):
                        for u in range(2):
                            _emit_body(nc, tc, u, AF, d["fst"], ct, out_d,
                                       generic_c, b1_nonzero, stop_after,
                                       pp_sh=pp_sh)
            else:
                for _body_i in range(n_bodies):
                    _emit_body(nc, tc, _body_i, AF, d["fst"], ct, out_d,
                               generic_c, b1_nonzero, stop_after)

    nc.compile()
    _CACHE[key] = nc
    return nc


def _emit_body(nc, tc, uid, AF, fst_d, ct, out_d, generic_c, b1_nonzero,
               stop_after=None, pp_sh=None):
        import contextlib
        idxc, idxn = ct["idxc"], ct["idxn"]
        maskc, maskn = ct["maskc"], ct["maskn"]
        wblk, w1qk, b0b = ct["wblk"], ct["w1qk"], ct["b0b"]
        ident, identb, ones128 = ct["ident"], ct["identb"], ct["ones128"]
        ebias = ct["ebias"]
        with contextlib.ExitStack() as _st:
            wp = _st.enter_context(tc.tile_pool(name=f"work{uid}", bufs=1))
            pp = pp_sh if pp_sh is not None else _st.enter_context(
                tc.tile_pool(name=f"psum{uid}", bufs=2,
                             space=bass.MemorySpace.PSUM))
            # ---- gathers: dma_gather of 512B pair rows, int16 pair ids,
            # sample i lands at [i % 128, i // 128, :].  n split over SWDGE
            # queues 0/2/3, c on queue 1.
            ncj = NJ if generic_c else 2
            gn = wp.tile([128, NJ * 256], F16)
            gn3 = gn[:].rearrange("p (j e) -> p j e", e=256)
            for js, je, q in ((0, 6, 0), (6, 11, 2), (11, 16, 3)):
                ni = (je - js) * 128
                nc.gpsimd.dma_gather(
                    gn3[:, js:je, :], fst_d,
                    idxn[:, js * 8:js * 8 + ni // 16],
                    ni, ni, 256, queue_num=q,
                )
            gc = wp.tile([128, ncj * 256], F16)
            gc3 = gc[:].rearrange("p (j e) -> p j e", e=256)
            nc.gpsimd.dma_gather(
                gc3, fst_d, idxc[:], ncj * 128, ncj * 128, 256, queue_num=1,
            )

            if stop_after == "gather":
                dummy = wp.tile([1, 2], F32)
                nc.vector.tensor_copy(dummy[:], gn[0:1, 0:2])
                nc.sync.dma_start(out_d, dummy[:])
                return

            # ---- parity select + diff (s-rows orientation, fp16) ----
            for j in range(ncj):
                mc = maskc[:, j:j + 1].to_broadcast([128, 128])
                nc.vector.copy_predicated(
                    gc3[:, j, 0:128], mc, gc3[:, j, 128:256])
            djall = wp.tile([128, S], F16)
            for j in range(NJ):
                mn = maskn[:, j:j + 1].to_broadcast([128, 128])
                nc.vector.copy_predicated(
                    gn3[:, j, 0:128], mn, gn3[:, j, 128:256])
                jc = j if generic_c else j % 2
                nc.vector.tensor_sub(
                    djall[:, ts(j, 128)], gc3[:, jc, 0:128], gn3[:, j, 0:128]
                )

            # ---- transpose diff blocks to [128ch, S] ----
            hin = wp.tile([128, S], BF16)
            for j2 in range(NJ // 2):
                pst = pp.tile([128, 256], F16, tag="ps")
                for h in range(2):
                    nc.tensor.transpose(
                        out=pst[:, ts(h, 128)],
                        in_=djall[:, ts(2 * j2 + h, 128)], identity=ident[:]
                    )
                if j2 % 2 == 0:
                    nc.vector.tensor_copy(hin[:, ts(j2, 256)], pst[:])
                else:
                    nc.scalar.copy(hin[:, ts(j2, 256)], pst[:])

            # ---- W0 matmul + relu (bias b0) ----
            hid = wp.tile([128, S], BF16)
            for j in range(4):
                psH = pp.tile([128, 512], F32, tag="ps")
                nc.tensor.matmul(
                    out=psH[:],
                    lhsT=wblk[:],
                    rhs=hin[:, ts(j, 512)],
                    start=True,
                    stop=True,
                )
                nc.scalar.activation(
                    hid[:, ts(j, 512)], psH[:], AF.Relu, bias=b0b[:, 0:1]
                )

            if stop_after == "transform":
                dummy = wp.tile([1, 2], F32)
                nc.vector.tensor_copy(dummy[:], hid[0:1, 0:2])
                nc.sync.dma_start(out_d, dummy[:])
                return

            # ---- MLP layer 2 + L2 normalize, transposed orientation ----
            # E^T tiles: [128 samples, 32] = [Eq^T | Ek^T] per 128-sample
            # block, all 16 blocks packed in one PSUM bank [128, 512].
            psET = pp.tile([128, 512], F32, tag="ps")
            if b1_nonzero:
                nc.tensor.matmul(
                    out=psET[:], lhsT=ct["onessq"][:], rhs=ct["b1w"][:],
                    start=True, stop=False,
                )
            for t in range(NJ):
                nc.tensor.matmul(
                    out=psET[:, ts(t, 32)],
                    lhsT=hid[:, ts(t, 128)],
                    rhs=w1qk[:],
                    start=not b1_nonzero,
                    stop=True,
                )
            et = wp.tile([128, 512], F32)
            nc.vector.tensor_copy(et[:], psET[:])
            sq = wp.tile([128, 512], F32)
            nc.vector.tensor_mul(sq[:], et[:], et[:])
            ss = wp.tile([128, 32], F32)
            nc.vector.tensor_reduce(
                ss[:].rearrange("p (t u) -> p t u", u=1),
                sq[:].rearrange("p (t c) -> p t c", c=16),
                axis=mybir.AxisListType.X, op=mybir.AluOpType.add,
            )
            nrm = wp.tile([128, 32], F32)
            nc.scalar.activation(nrm[:], ss[:], AF.Sqrt)
            nrme = wp.tile([128, 32], F32)
            nc.vector.tensor_scalar_add(nrme[:], nrm[:], EPS)
            inv = wp.tile([128, 32], F32)
            nc.vector.reciprocal_approx_fast(inv[:], nrme[:])
            fT = wp.tile([128, 512], BF16)
            nc.vector.tensor_mul(
                fT[:].rearrange("p (t c) -> p t c", c=16),
                et[:].rearrange("p (t c) -> p t c", c=16),
                inv[:].to_broadcast([128, 32, 16]),
            )

            # ---- l_pos partials: sum_c Fq*Fk per sample ----
            fT4 = fT[:].rearrange("p (t two c) -> p t two c", two=2, c=16)
            prod = wp.tile([128, 256], F32)
            nc.vector.tensor_mul(
                prod[:].rearrange("p (t c) -> p t c", c=16),
                fT4[:, :, 0, :], fT4[:, :, 1, :],
            )
            lpost = wp.tile([128, 16], F32)
            nc.vector.tensor_reduce(
                lpost[:].rearrange("p (t u) -> p t u", u=1),
                prod[:].rearrange("p (t c) -> p t c", c=16),
                axis=mybir.AxisListType.X, op=mybir.AluOpType.add,
            )
            lred = wp.tile([128, 2], F32)
            nc.vector.tensor_reduce(
                lred[:, 1:2], lpost[:],
                axis=mybir.AxisListType.X, op=mybir.AluOpType.add,
            )

            # ---- transpose F^T back to [16, S] bf16 for the NCE ----
            fqb = wp.tile([16, S], BF16)
            fkb = wp.tile([16, S], BF16)
            for half, fb in ((0, fqb), (1, fkb)):
                psF = pp.tile([16, S], BF16, tag="ps")
                for t in range(NJ):
                    nc.tensor.transpose(
                        out=psF[:, ts(t, 128)],
                        in_=fT[:, ds(t * 32 + half * 16, 16)],
                        identity=identb[:],
                    )
                if half == 0:
                    nc.vector.tensor_copy(fb[:], psF[:])
                else:
                    nc.scalar.copy(fb[:], psF[:])

            out_sb = wp.tile([1, 2], F32)

            if stop_after == "mlp":
                nc.vector.tensor_copy(out_sb[:], fqb[0:1, 0:2])
                nc.sync.dma_start(out_d, out_sb[:])
                return

            # ---- NCE: 16 row-chunks of M, exp+rowsum fused ----
            rowsums = wp.tile([128, 16], F32)
            escr = wp.tile([128, S], BF16)
            for i in range(16):
                psM = pp.tile([128, S], F32, tag="ps")
                for j in range(4):
                    nc.tensor.matmul(
                        out=psM[:, ts(j, 512)],
                        lhsT=fqb[:, ts(i, 128)],
                        rhs=fkb[:, ts(j, 512)],
                        start=True,
                        stop=True,
                    )
                if i % 8 in (0, 3, 6):  # 6 direct, 10 offloaded: ACT~DVE balance
                    # direct: ACT reads PSUM f32 (1x)
                    nc.scalar.activation(
                        escr[:], psM[:], AF.Exp,
                        bias=ebias[:, 0:1], scale=1.0 / TAU,
                        accum_out=rowsums[:, i:i + 1],
                    )
                else:
                    # offload PSUM read to DVE; ACT exp runs 2x from bf16
                    # SBUF; double-buffered so DVE copy i+1 overlaps exp i
                    msb = wp.tile([128, S], BF16, tag=f"msb{i % 2}")
                    nc.vector.tensor_copy(msb[:], psM[:])
                    nc.scalar.activation(
                        escr[:], msb[:], AF.Exp,
                        bias=ebias[:, 0:1], scale=1.0 / TAU,
                        accum_out=rowsums[:, i:i + 1],
                    )

            logt = wp.tile([128, 16], F32)
            nc.scalar.activation(logt[:], rowsums[:], AF.Ln)
            nc.vector.tensor_reduce(
                lred[:, 0:1], logt[:], axis=mybir.AxisListType.X,
                op=mybir.AluOpType.add,
            )
            psS = pp.tile([1, 2], F32, tag="ps")
            nc.tensor.matmul(
                out=psS[:], lhsT=ones128[:], rhs=lred[:], start=True, stop=True
            )
            nc.vector.tensor_copy(out_sb[:], psS[:])
            nc.sync.dma_start(out_d, out_sb[:])


def _host_prep(f_q, f_k, W0, b0, W1, b1, c_ids, n_ids):
    """Build the per-core input maps (host-side sharding + layout prep)."""
    f_q = np.asarray(f_q, dtype=np.float32).reshape(B, C, HW)
    f_k = np.asarray(f_k, dtype=np.float32).reshape(B, C, HW)
    W0 = np.asarray(W0, dtype=np.float32)
    b0 = np.asarray(b0, dtype=np.float32)
    W1 = np.asarray(W1, dtype=np.float32)
    b1 = np.asarray(b1, dtype=np.float32)
    c_ids = np.asarray(c_ids).astype(np.int64)
    n_ids = np.asarray(n_ids).astype(np.int64)

    generic_c = not np.array_equal(np.tile(c_ids[:NU], 8), c_ids)
    b1_nonzero = bool(np.any(b1 != 0))

    import ml_dtypes
    bf = ml_dtypes.bfloat16
    wblk = np.zeros((128, 128), np.float32)
    wblk[0:64, 0:64] = W0
    wblk[64:128, 64:128] = W0
    wblk = wblk.astype(bf)
    w1qk = np.zeros((128, 32), np.float32)
    w1qk[0:64, 0:16] = W1
    w1qk[64:128, 16:32] = W1
    w1qk = w1qk.astype(bf)
    b0b = np.concatenate([b0, b0]).reshape(128, 1).astype(np.float32)

    def wrap16(ids):
        # dma_gather idx layout: idxs[p, s] = pair_id[s*16 + p] for p < 16,
        # replicated across the 8 partition groups of 16
        w = (ids >> 1).astype(np.int16).reshape(-1, 16).T
        return np.tile(w, (8, 1)).copy()

    def parity(ids, nj):
        # m[p, j] = odd-parity of sample s = j*128 + p
        return (ids & 1).astype(np.int8).reshape(nj, 128).T.copy()

    c_eff = c_ids if generic_c else c_ids[:NU]
    common = {
        "wblk": wblk, "w1qk": w1qk, "b0b": b0b,
        "ones128": np.ones((128, 1), np.float32),
        "ident": np.eye(128, dtype=np.float16),
        "identb": np.eye(128, dtype=np.float32).astype(bf),
        "idxn": wrap16(n_ids), "idxc": wrap16(c_eff),
        "maskn": parity(n_ids, NJ),
        "maskc": parity(c_eff, NJ if generic_c else 2),
    }
    if b1_nonzero:
        common["onessq"] = np.ones((128, 128), np.float32).astype(bf)
        b1p = np.zeros((32,), np.float32)
        b1p[0:16] = b1
        b1p[16:32] = b1
        common["b1w"] = np.tile(b1p / 128.0, 16).reshape(1, 512).repeat(
            128, axis=0).astype(bf)

    in_maps = []
    for b in range(B):
        m = dict(common)
        # [HW/2, 256] fp16: row pid = [fq|fk of px 2*pid, fq|fk of
        # 2*pid+1] — one 512B row per pixel pair (pair id fits int16).
        fst = np.empty((HW, 128), np.float16)
        fst[:, 0:64] = f_q[b].T
        fst[:, 64:128] = f_k[b].T
        m["fst"] = fst.reshape(HW // 2, 256)
        in_maps.append(m)
    return in_maps, generic_c, b1_nonzero


def _finish(results):
    total = 0.0
    for r in results:
        o = np.asarray(r["out"], dtype=np.float64).reshape(2)
        total += S / TAU + o[0] - o[1] / TAU
    return np.float32(total / (B * S))


def kernel(**inputs) -> np.ndarray:
    in_maps, generic_c, b1_nonzero = _host_prep(
        inputs["f_q"], inputs["f_k"], inputs["W0"], inputs["b0"],
        inputs["W1"], inputs["b1"], inputs["c_ids"], inputs["n_ids"],
    )
    nc = _build(generic_c=generic_c, b1_nonzero=b1_nonzero)
    res = bass_utils.run_bass_kernel_spmd(
        nc, in_maps, core_ids=list(range(NCORES))
    )
    return _finish(res.results)
